# revision 1
# baseline (speedup 1.0000x reference)
"""Trainium2 Bass kernel for a 2-layer GCN (PyG GCNConv semantics).

Full-input contract: kernel(**inputs) takes the complete (unsharded) numpy
inputs and returns the full [N, OUT] float32 output.  Internally the nodes are
sharded across 8 NeuronCores (1D graph partition); transformed node features
are AllGathered between layers; edge aggregation runs as
dma_gather + one-hot-selection-matrix matmuls on each core.

Math:  h  = relu(Ahat @ (x @ W1) + b1)
       out =       Ahat @  h      @ W2 + b2   (computed as (Ahat @ h) @ W2 + b2)
with Ahat = D^-1/2 (A + I) D^-1/2, deg counted at target incl. self-loop.
"""

import math

import numpy as np
import ml_dtypes

BF16 = ml_dtypes.bfloat16

# ---- hardcoded problem constants (see module docstring) ----
N_FULL = 50000
F_IN = 256
HID = 128
OUT = 64
NCORES = 8
P = 128
HALFSPLIT = 32768  # int16 index limit for dma_gather
CHUNK = 32         # gather-call granularity in 128-edge tiles (4096 rows/call; >6k rows wedges SWDGE)


# ----------------------------------------------------------------------------
# host-side graph preprocessing
# ----------------------------------------------------------------------------

def _balance_nodes(n, nshard, nblk, weight):
    """Assign nodes to ncores*nblk blocks (block b of each core has capacity
    128, except each core's last block has nshard-(nblk-1)*128) with LPT greedy
    so per-block weight sums equalize.  Returns perm (block-concatenated node
    order) and pos[node] -> position in that order."""
    import heapq

    lastcap = nshard - (nblk - 1) * P
    nb = NCORES * nblk
    caps = np.full(nb, P, np.int64)
    caps[nblk - 1 :: nblk] = lastcap
    order = np.argsort(-weight, kind="stable")
    heap = [(0.0, int(b)) for b in range(nb)]
    heapq.heapify(heap)
    members = [[] for _ in range(nb)]
    spill = []
    for node in order:
        while True:
            w, b = heapq.heappop(heap)
            if len(members[b]) < caps[b]:
                members[b].append(node)
                if len(members[b]) < caps[b]:
                    heapq.heappush(heap, (w + weight[node], b))
                break
            # full block: drop from heap
            if not heap:
                spill.append(node)
                break
        if not heap and spill:
            break
    assert not spill
    perm = np.concatenate([np.asarray(m, np.int64) for m in members])
    pos = np.empty(n, np.int64)
    pos[perm] = np.arange(n)
    return perm, pos


def host_prep(edge_index, n, ncores, halfsplit, chunk):
    """Sort/pad/shard edges.  Returns static structure (shared by all cores)
    and per-core index/metadata arrays."""
    nshard = n // ncores
    nblk = _ceil(nshard, P)

    src = np.asarray(edge_index[0], np.int64)
    dst = np.asarray(edge_index[1], np.int64)
    loops = np.arange(n, dtype=np.int64)
    src_a = np.concatenate([src, loops])
    dst_a = np.concatenate([dst, loops])
    deg = np.bincount(dst_a, minlength=n).astype(np.float64)  # incl self-loop
    dinv = 1.0 / np.sqrt(deg)
    norm = (dinv[src_a] * dinv[dst_a]).astype(np.float32)

    # balanced node->(core,block,slot) assignment by aggregation work (in-deg)
    perm, pos = _balance_nodes(n, nshard, nblk, deg)

    psrc = pos[src_a]
    pdst = pos[dst_a]
    core = pdst // nshard
    dstl = pdst - core * nshard
    blk = dstl // P
    slot = dstl - blk * P
    half = (psrc >= halfsplit).astype(np.int64)

    key = (core * 2 + half) * nblk + blk
    counts = np.bincount(key, minlength=ncores * 2 * nblk).reshape(ncores, 2, nblk)
    tiles_hb = np.ceil(counts.max(axis=0) / P).astype(np.int64)  # [2, nblk]

    order = np.argsort(key, kind="stable")
    s_src = psrc[order]
    s_norm = norm[order]
    s_slot = slot[order]

    # group offsets in sorted stream, (core, half, blk) order
    goff = np.zeros(ncores * 2 * nblk + 1, np.int64)
    np.cumsum(counts.reshape(-1), out=goff[1:])

    # global padded tile layout: half-primary, then block
    tile_start = np.zeros((2, nblk), np.int64)
    t = 0
    for h in (0, 1):
        for b in range(nblk):
            tile_start[h, b] = t
            t += tiles_hb[h, b]
    t_total = int(t)

    idx_flat = np.zeros((ncores, t_total * P), np.int16)
    slot_flat = np.zeros((ncores, t_total * P), np.float32)
    norm_flat = np.zeros((ncores, t_total * P), np.float32)
    for c in range(ncores):
        for h in (0, 1):
            for b in range(nblk):
                g0 = goff[(c * 2 + h) * nblk + b]
                cnt = counts[c, h, b]
                o0 = tile_start[h, b] * P
                idx_flat[c, o0 : o0 + cnt] = (s_src[g0 : g0 + cnt] - h * halfsplit).astype(np.int16)
                slot_flat[c, o0 : o0 + cnt] = s_slot[g0 : g0 + cnt]
                norm_flat[c, o0 : o0 + cnt] = s_norm[g0 : g0 + cnt]

    # wrapped int16 index layout: entry j of the stream -> [j%16, j//16],
    # replicated across the 8 Q7 partition groups
    idx_w = np.empty((ncores, 128, t_total * 8), np.int16)
    meta = np.empty((ncores, 128, 2 * t_total), np.float32)
    for c in range(ncores):
        w = idx_flat[c].reshape(-1, 16).T  # [16, t_total*8]
        idx_w[c] = np.tile(w, (8, 1))
        meta[c, :, 0::2] = slot_flat[c].reshape(t_total, P).T
        meta[c, :, 1::2] = norm_flat[c].reshape(t_total, P).T

    # gather-call chunks (within each half; spans of <= chunk tiles)
    chunks = []
    for h in (0, 1):
        h0 = int(tile_start[h, 0])
        hcnt = int(tiles_hb[h].sum())
        s = h0
        while s < h0 + hcnt:
            cnt = min(chunk, h0 + hcnt - s)
            chunks.append((h, s, cnt))
            s += cnt

    # per-tile info: (half, blk, first_in_group, last_in_group)
    tile_info = []
    for h in (0, 1):
        for b in range(nblk):
            nt = int(tiles_hb[h, b])
            for i in range(nt):
                tile_info.append((h, b, i == 0, i == nt - 1))
    assert len(tile_info) == t_total

    struct = dict(
        n=n, ncores=ncores, nshard=nshard, nblk=nblk,
        lastv=nshard - (nblk - 1) * P, npad=nblk * P,
        t_total=t_total, tiles_hb=tiles_hb, tile_start=tile_start,
        chunks=chunks, tile_info=tile_info, halfsplit=halfsplit,
    )
    percore = dict(idx_w=idx_w, meta=meta)
    return struct, percore, perm, pos


def _ceil(a, b):
    return -(-a // b)


# ----------------------------------------------------------------------------
# device program
# ----------------------------------------------------------------------------

def build_program(st, f_in, hid, out_f, has_b1, has_b2, enable_asserts=False, reps=1, pm=None):
    pm = dict({"A": 1, "AGZ": 1, "B": 1, "AGH": 1, "C": 1, "D": 1}, **(pm or {}))
    import concourse.bass as bass
    import concourse.mybir as mybir
    import concourse.tile as tile
    from concourse import bacc

    dt = mybir.dt
    Alu = mybir.AluOpType
    Act = mybir.ActivationFunctionType

    ncores = st["ncores"]
    nshard, nblk, npad, lastv = st["nshard"], st["nblk"], st["npad"], st["lastv"]
    t_total = st["t_total"]
    chunks = st["chunks"]
    tile_info = st["tile_info"]
    tiles_hb = st["tiles_hb"]
    halfsplit = st["halfsplit"]
    kt = f_in // P  # k-tiles for layer-1 dense

    nc = bacc.Bacc(
        "TRN2", target_bir_lowering=False, debug=False,
        enable_asserts=enable_asserts, num_devices=ncores,
    )

    # ---- I/O ----
    xT_d = nc.dram_tensor("xT", [P, kt, npad], dt.bfloat16, kind="ExternalInput")
    w1_d = nc.dram_tensor("w1", [P, kt, hid], dt.bfloat16, kind="ExternalInput")
    w2_d = nc.dram_tensor("w2", [hid, out_f], dt.bfloat16, kind="ExternalInput")
    iota_d = nc.dram_tensor("iota", [P, P], dt.bfloat16, kind="ExternalInput")
    idx_d = nc.dram_tensor("idx", [128, t_total * 8], dt.int16, kind="ExternalInput")
    meta_d = nc.dram_tensor("meta", [128, 2 * t_total], dt.float32, kind="ExternalInput")
    if has_b1:
        b1_d = nc.dram_tensor("b1bc", [P, hid], dt.float32, kind="ExternalInput")
    if has_b2:
        b2_d = nc.dram_tensor("b2bc", [P, out_f], dt.float32, kind="ExternalInput")
    out_d = nc.dram_tensor("out", [nshard, out_f], dt.float32, kind="ExternalOutput")

    rg = [list(range(ncores))]

    with tile.TileContext(nc) as tc:
        with (
            tc.tile_pool(name="const", bufs=1) as constp,
            tc.tile_pool(name="stage", bufs=1) as stagep,
            tc.tile_pool(name="dram", bufs=1, space="DRAM") as dramp,
            tc.tile_pool(name="gpool", bufs=2) as gpool,
            tc.tile_pool(name="spool", bufs=16) as spool,
        ):
            # ---- persistent SBUF ----
            xT_sb = constp.tile([P, kt, npad], dt.bfloat16)
            w1_sb = constp.tile([P, kt, hid], dt.bfloat16)
            w2_sb = constp.tile([hid, out_f], dt.bfloat16)
            iota_sb = constp.tile([P, P], dt.bfloat16)
            idx_sb = constp.tile([128, t_total * 8], dt.int16)
            meta_sb = constp.tile([128, 2 * t_total], dt.float32)
            nc.sync.dma_start(out=xT_sb[:], in_=xT_d[:])
            nc.sync.dma_start(out=w1_sb[:], in_=w1_d[:])
            nc.sync.dma_start(out=w2_sb[:], in_=w2_d[:])
            nc.sync.dma_start(out=iota_sb[:], in_=iota_d[:])
            nc.sync.dma_start(out=idx_sb[:], in_=idx_d[:])
            nc.sync.dma_start(out=meta_sb[:], in_=meta_d[:])
            if has_b1:
                b1_sb = constp.tile([P, hid], dt.float32)
                nc.sync.dma_start(out=b1_sb[:], in_=b1_d[:])
            if has_b2:
                b2_sb = constp.tile([P, out_f], dt.float32)
                nc.sync.dma_start(out=b2_sb[:], in_=b2_d[:])

            zstage = stagep.tile([P, npad], dt.bfloat16)    # z = x@W1, node-major
            acc = stagep.tile([P, npad], dt.float32)        # f32 accumulator (both layers)
            hstage = stagep.tile([P, npad], dt.bfloat16)    # relu'd h, node-major
            aggT = stagep.tile([P, npad], dt.bfloat16)      # L2 agg, feat-major
            outstage = stagep.tile([P, nblk * out_f], dt.float32)


            def bts(i, sz):  # block tile slice
                return slice(i * sz, (i + 1) * sz)

            def valid(b):
                return lastv if b == nblk - 1 else P

            for _rep in range(reps):
                # ---- internal DRAM (gather tables / collective buffers) ----
                z_loc = dramp.tile([nshard, hid], dt.bfloat16, name=f"z_loc{_rep}")
                h_loc = dramp.tile([nshard, hid], dt.bfloat16, name=f"h_loc{_rep}")

                # ================= phase A: z = x @ W1 (node-major) =================
                for _ra in range(pm["A"]):
                  with tc.tile_pool(name=f"pA{_rep}_{_ra}", bufs=4, space="PSUM") as pA:
                    for t in range(nblk):
                        ps = pA.tile([P, hid], dt.float32, tag="psA")
                        for k in range(kt):
                            nc.tensor.matmul(
                                out=ps[:], lhsT=xT_sb[:, k, bts(t, P)], rhs=w1_sb[:, k, :],
                                start=(k == 0), stop=(k == kt - 1),
                            )
                        nc.scalar.copy(out=zstage[:, bts(t, hid)], in_=ps[:])
                        v = valid(t)
                        nc.sync.dma_start(out=z_loc[t * P : t * P + v, :], in_=zstage[:v, bts(t, hid)])

                for _rz in range(pm["AGZ"]):
                    z_full = dramp.tile([st["n"], hid], dt.bfloat16, addr_space="Shared",
                                        name=f"z_full{_rep}_{_rz}")
                    nc.gpsimd.collective_compute(
                        "AllGather", mybir.AluOpType.bypass, replica_groups=rg,
                        ins=[z_loc[:]], outs=[z_full[:]],
                    )

                # ================= edge aggregation (shared helper) =================
                def aggregate(layer, table, sub=0):
                    """layer 1: psum[slot,feat] (lhsT=S, rhs=G);
                    layer 2: psum[feat,slot] (lhsT=G, rhs=S).  Accumulate across the
                    two src-halves into `acc` (f32)."""
                    with tc.tile_pool(name=f"pB{layer}_{_rep}_{sub}", bufs=8, space="PSUM") as pB:
                        psd = {}
                        for (h, cstart, cnt) in chunks:
                            g = gpool.tile([P, CHUNK, hid], dt.bfloat16, tag="g", name=f"g{layer}")
                            src_ap = table[:] if h == 0 else table[halfsplit:, :]
                            nc.gpsimd.dma_gather(
                                g[:, :cnt, :], src_ap,
                                idx_sb[:, cstart * 8 : (cstart + cnt) * 8],
                                cnt * P, cnt * P, hid, single_packet=False,
                            )
                            for p in range(cnt):
                                t = cstart + p
                                th, b, first, last = tile_info[t]
                                S = spool.tile([P, P], dt.bfloat16, tag="s", name=f"s{layer}")
                                nc.vector.tensor_scalar(
                                    out=S[:], in0=iota_sb[:],
                                    scalar1=meta_sb[:, 2 * t : 2 * t + 1],
                                    scalar2=meta_sb[:, 2 * t + 1 : 2 * t + 2],
                                    op0=Alu.is_equal, op1=Alu.mult,
                                )
                                if first:
                                    psd[b] = pB.tile([P, P], dt.float32, tag="psB", name=f"ps{layer}")
                                if layer == 1:
                                    nc.tensor.matmul(out=psd[b][:, :hid], lhsT=S[:], rhs=g[:, p, :],
                                                     start=first, stop=last)
                                else:
                                    nc.tensor.matmul(out=psd[b][:], lhsT=g[:, p, :], rhs=S[:],
                                                     start=first, stop=last)
                                if last:
                                    if th == 0 or tiles_hb[0, b] == 0:
                                        nc.scalar.copy(out=acc[:, bts(b, P)], in_=psd[b][:])
                                    else:
                                        nc.vector.tensor_tensor(
                                            out=acc[:, bts(b, P)], in0=psd[b][:],
                                            in1=acc[:, bts(b, P)], op=Alu.add,
                                        )
                                    del psd[b]

                # ================= phase B: L1 aggregation + relu =================
                for _rb in range(pm["B"]):
                    aggregate(1, z_full, sub=_rb)
                    for b in range(nblk):
                        if has_b1:
                            nc.vector.tensor_tensor(out=acc[:, bts(b, P)], in0=acc[:, bts(b, P)],
                                                    in1=b1_sb[:], op=Alu.add)
                        nc.scalar.activation(out=hstage[:, bts(b, P)], in_=acc[:, bts(b, P)], func=Act.Relu)
                        v = valid(b)
                        nc.sync.dma_start(out=h_loc[b * P : b * P + v, :], in_=hstage[:v, bts(b, P)])

                for _rh in range(pm["AGH"]):
                    h_full = dramp.tile([st["n"], hid], dt.bfloat16, addr_space="Shared",
                                        name=f"h_full{_rep}_{_rh}")
                    nc.gpsimd.collective_compute(
                        "AllGather", mybir.AluOpType.bypass, replica_groups=rg,
                        ins=[h_loc[:]], outs=[h_full[:]],
                    )

                # ================= phase C: L2 aggregation (feat-major) =============
                for _rc in range(pm["C"]):
                    aggregate(2, h_full, sub=100 + _rc)
                    for b in range(nblk):
                        nc.scalar.copy(out=aggT[:, bts(b, P)], in_=acc[:, bts(b, P)])

                # ================= phase D: out = agg @ W2 (+ b2) ==================
                for _rd in range(pm["D"]):
                  with tc.tile_pool(name=f"pD{_rep}_{_rd}", bufs=4, space="PSUM") as pD:
                    for t in range(nblk):
                        ps = pD.tile([P, out_f], dt.float32, tag="psD")
                        nc.tensor.matmul(out=ps[:], lhsT=aggT[:, bts(t, P)], rhs=w2_sb[:],
                                         start=True, stop=True)
                        if has_b2:
                            nc.vector.tensor_tensor(out=outstage[:, bts(t, out_f)], in0=ps[:],
                                                    in1=b2_sb[:], op=Alu.add)
                        else:
                            nc.scalar.copy(out=outstage[:, bts(t, out_f)], in_=ps[:])
                        v = valid(t)
                        nc.sync.dma_start(out=out_d[t * P : t * P + v, :],
                                          in_=outstage[:v, bts(t, out_f)])

    nc.compile()
    return nc


# ----------------------------------------------------------------------------
# input packing
# ----------------------------------------------------------------------------

def pack_inputs(x, W1, b1, W2, b2, st, percore, perm):
    ncores, nshard, npad = st["ncores"], st["nshard"], st["npad"]
    kt = x.shape[1] // P
    hid = W1.shape[1]
    out_f = W2.shape[1]
    has_b1 = bool(np.any(b1))
    has_b2 = bool(np.any(b2))

    w1h = np.ascontiguousarray(
        W1.reshape(kt, P, hid).transpose(1, 0, 2)).astype(BF16)
    w2h = np.ascontiguousarray(W2).astype(BF16)
    iota_h = np.broadcast_to(np.arange(P, dtype=np.float32), (P, P)).astype(BF16)
    iota_h = np.ascontiguousarray(iota_h)

    xp = x[perm]  # balanced node order
    in_maps = []
    for c in range(ncores):
        xpad = np.zeros((npad, kt * P), np.float32)
        xpad[:nshard] = xp[c * nshard : (c + 1) * nshard]
        xT = np.ascontiguousarray(
            xpad.T.reshape(kt, P, npad).transpose(1, 0, 2)).astype(BF16)
        m = {
            "xT": xT, "w1": w1h, "w2": w2h, "iota": iota_h,
            "idx": np.ascontiguousarray(percore["idx_w"][c]),
            "meta": np.ascontiguousarray(percore["meta"][c]),
        }
        if has_b1:
            m["b1bc"] = np.ascontiguousarray(np.broadcast_to(b1, (P, hid))).astype(np.float32)
        if has_b2:
            m["b2bc"] = np.ascontiguousarray(np.broadcast_to(b2, (P, out_f))).astype(np.float32)
        in_maps.append(m)
    return in_maps, has_b1, has_b2


# ----------------------------------------------------------------------------
# entry point
# ----------------------------------------------------------------------------

_CACHE = {}


def _run(x, edge_index, W1, b1, W2, b2, trace=False):
    from concourse.bass_utils import run_bass_kernel_spmd

    n = x.shape[0]
    st, percore, perm, pos = host_prep(edge_index, n, NCORES, HALFSPLIT, CHUNK)
    in_maps, has_b1, has_b2 = pack_inputs(x, W1, b1, W2, b2, st, percore, perm)

    key = (n, x.shape[1], W1.shape[1], W2.shape[1], st["t_total"],
           tuple(st["tiles_hb"].reshape(-1)), has_b1, has_b2)
    nc = _CACHE.get(key)
    if nc is None:
        nc = build_program(st, x.shape[1], W1.shape[1], W2.shape[1], has_b1, has_b2)
        _CACHE[key] = nc

    res = run_bass_kernel_spmd(nc, in_maps, core_ids=list(range(NCORES)), trace=trace)
    outp = np.concatenate([res.results[c]["out"] for c in range(NCORES)], axis=0)
    out = np.empty_like(outp)
    out[perm] = outp  # undo balanced permutation
    return out.astype(np.float32), res


def kernel(x, edge_index, W1, b1, W2, b2):
    out, _ = _run(np.asarray(x, np.float32), np.asarray(edge_index),
                  np.asarray(W1, np.float32), np.asarray(b1, np.float32),
                  np.asarray(W2, np.float32), np.asarray(b2, np.float32))
    return out



# revision 9
# speedup vs baseline: 1.5941x; 1.5941x over previous
"""Trainium2 Bass kernel for a 2-layer GCN (PyG GCNConv semantics).

Full-input contract: kernel(**inputs) takes the complete (unsharded) numpy
inputs and returns the full [N, OUT] float32 output.  Internally the nodes are
sharded across 8 NeuronCores (1D graph partition); transformed node features
are AllGathered between layers; edge aggregation runs as
dma_gather + one-hot-selection-matrix matmuls on each core.  The one-hot
selection matrices are precomputed on the host and streamed from DRAM (they
are identical for both layers), keeping the vector engine idle.

Math:  h  = relu(Ahat @ (x @ W1) + b1)
       out =       Ahat @  h      @ W2 + b2   (computed as (Ahat @ h) @ W2 + b2)
with Ahat = D^-1/2 (A + I) D^-1/2, deg counted at target incl. self-loop.
"""

import math

import numpy as np
import ml_dtypes

BF16 = ml_dtypes.bfloat16

# ---- hardcoded problem constants (see module docstring) ----
N_FULL = 50000
F_IN = 256
HID = 128
OUT = 64
NCORES = 8
P = 128
HALFSPLIT = 32768  # int16 index limit for dma_gather
CHUNK = 32         # gather-call granularity in 128-edge tiles (4096 rows/call; >6k rows wedges SWDGE)


# ----------------------------------------------------------------------------
# host-side graph preprocessing
# ----------------------------------------------------------------------------

def _balance_nodes(n, nshard, nblk, weight):
    """Assign nodes to ncores*nblk blocks (block b of each core has capacity
    128, except each core's last block has nshard-(nblk-1)*128) with LPT greedy
    so per-block weight sums equalize.  Returns perm (block-concatenated node
    order) and pos[node] -> position in that order."""
    import heapq

    lastcap = nshard - (nblk - 1) * P
    nb = NCORES * nblk
    caps = np.full(nb, P, np.int64)
    caps[nblk - 1 :: nblk] = lastcap
    order = np.argsort(-weight, kind="stable")
    heap = [(0.0, int(b)) for b in range(nb)]
    heapq.heapify(heap)
    members = [[] for _ in range(nb)]
    spill = []
    for node in order:
        while True:
            w, b = heapq.heappop(heap)
            if len(members[b]) < caps[b]:
                members[b].append(node)
                if len(members[b]) < caps[b]:
                    heapq.heappush(heap, (w + weight[node], b))
                break
            # full block: drop from heap
            if not heap:
                spill.append(node)
                break
        if not heap and spill:
            break
    assert not spill
    perm = np.concatenate([np.asarray(m, np.int64) for m in members])
    pos = np.empty(n, np.int64)
    pos[perm] = np.arange(n)
    return perm, pos


def host_prep(edge_index, n, ncores, halfsplit, chunk):
    """Sort/pad/shard edges.  Returns static structure (shared by all cores)
    and per-core index/selection-matrix arrays."""
    nshard = n // ncores
    nblk = _ceil(nshard, P)

    src = np.asarray(edge_index[0], np.int64)
    dst = np.asarray(edge_index[1], np.int64)
    loops = np.arange(n, dtype=np.int64)
    src_a = np.concatenate([src, loops])
    dst_a = np.concatenate([dst, loops])
    deg = np.bincount(dst_a, minlength=n).astype(np.float64)  # incl self-loop
    dinv = 1.0 / np.sqrt(deg)
    norm = (dinv[src_a] * dinv[dst_a]).astype(np.float32)

    # balanced node->(core,block,slot) assignment by aggregation work (in-deg)
    perm, pos = _balance_nodes(n, nshard, nblk, deg)

    psrc = pos[src_a]
    pdst = pos[dst_a]
    core = pdst // nshard
    dstl = pdst - core * nshard
    blk = dstl // P
    slot = dstl - blk * P
    half = (psrc >= halfsplit).astype(np.int64)

    key = (core * 2 + half) * nblk + blk
    counts = np.bincount(key, minlength=ncores * 2 * nblk).reshape(ncores, 2, nblk)
    tiles_hb = np.ceil(counts.max(axis=0) / P).astype(np.int64)  # [2, nblk]

    order = np.argsort(key, kind="stable")
    s_src = psrc[order]
    s_norm = norm[order]
    s_slot = slot[order]

    # group offsets in sorted stream, (core, half, blk) order
    goff = np.zeros(ncores * 2 * nblk + 1, np.int64)
    np.cumsum(counts.reshape(-1), out=goff[1:])

    # global padded tile layout: half-primary, then block
    tile_start = np.zeros((2, nblk), np.int64)
    t = 0
    for h in (0, 1):
        for b in range(nblk):
            tile_start[h, b] = t
            t += tiles_hb[h, b]
    t_total = int(t)

    idx_flat = np.zeros((ncores, t_total * P), np.int16)
    slot_flat = np.zeros((ncores, t_total * P), np.int64)
    norm_flat = np.zeros((ncores, t_total * P), np.float32)
    for c in range(ncores):
        for h in (0, 1):
            for b in range(nblk):
                g0 = goff[(c * 2 + h) * nblk + b]
                cnt = counts[c, h, b]
                o0 = tile_start[h, b] * P
                idx_flat[c, o0 : o0 + cnt] = (s_src[g0 : g0 + cnt] - h * halfsplit).astype(np.int16)
                slot_flat[c, o0 : o0 + cnt] = s_slot[g0 : g0 + cnt]
                norm_flat[c, o0 : o0 + cnt] = s_norm[g0 : g0 + cnt]

    # wrapped int16 index layout: entry j of the stream -> [j%16, j//16],
    # replicated across the 8 Q7 partition groups
    idx_w = np.empty((ncores, 128, t_total * 8), np.int16)
    # host-built one-hot selection matrices: S[e, t*128 + slot] = norm
    # (partition = row-in-tile e, free = (tile, slot)); shared by both layers
    smat = np.zeros((ncores, 128, t_total, 128), BF16)
    ar = np.arange(t_total * P)
    for c in range(ncores):
        w = idx_flat[c].reshape(-1, 16).T  # [16, t_total*8]
        idx_w[c] = np.tile(w, (8, 1))
        nz = norm_flat[c] != 0
        smat[c, ar[nz] % P, ar[nz] // P, slot_flat[c][nz]] = norm_flat[c][nz].astype(BF16)
    smat = smat.reshape(ncores, 128, t_total * 128)

    # gather-call chunks (within each half; spans of <= chunk tiles)
    chunks = []
    for h in (0, 1):
        h0 = int(tile_start[h, 0])
        hcnt = int(tiles_hb[h].sum())
        s = h0
        while s < h0 + hcnt:
            cnt = min(chunk, h0 + hcnt - s)
            chunks.append((h, s, cnt))
            s += cnt

    # per-tile info: (half, blk, first_in_group, last_in_group)
    tile_info = []
    for h in (0, 1):
        for b in range(nblk):
            nt = int(tiles_hb[h, b])
            for i in range(nt):
                tile_info.append((h, b, i == 0, i == nt - 1))
    assert len(tile_info) == t_total

    struct = dict(
        n=n, ncores=ncores, nshard=nshard, nblk=nblk,
        lastv=nshard - (nblk - 1) * P, npad=nblk * P,
        t_total=t_total, tiles_hb=tiles_hb, tile_start=tile_start,
        chunks=chunks, tile_info=tile_info, halfsplit=halfsplit,
    )
    percore = dict(idx_w=idx_w, smat=smat)
    return struct, percore, perm, pos


def _ceil(a, b):
    return -(-a // b)


# ----------------------------------------------------------------------------
# device program
# ----------------------------------------------------------------------------

def build_program(st, f_in, hid, out_f, has_b1, has_b2, enable_asserts=False, reps=1, pm=None,
                  nqueues=4):
    pm = dict({"A": 1, "AGZ": 1, "B": 1, "AGH": 1, "C": 1, "D": 1}, **(pm or {}))
    import concourse.bass as bass
    import concourse.mybir as mybir
    import concourse.tile as tile
    from concourse import bacc

    dt = mybir.dt
    Alu = mybir.AluOpType
    Act = mybir.ActivationFunctionType

    ncores = st["ncores"]
    nshard, nblk, npad, lastv = st["nshard"], st["nblk"], st["npad"], st["lastv"]
    t_total = st["t_total"]
    chunks = st["chunks"]
    tile_info = st["tile_info"]
    tiles_hb = st["tiles_hb"]
    halfsplit = st["halfsplit"]
    kt = f_in // P  # k-tiles for layer-1 dense

    nc = bacc.Bacc(
        "TRN2", target_bir_lowering=False, debug=False,
        enable_asserts=enable_asserts, num_devices=ncores,
        num_swdge_queues=nqueues,
    )

    # ---- I/O ----
    xT_d = nc.dram_tensor("xT", [P, kt, npad], dt.bfloat16, kind="ExternalInput")
    w1_d = nc.dram_tensor("w1", [P, kt, hid], dt.bfloat16, kind="ExternalInput")
    w2_d = nc.dram_tensor("w2", [hid, out_f], dt.bfloat16, kind="ExternalInput")
    idx_d = nc.dram_tensor("idx", [128, t_total * 8], dt.int16, kind="ExternalInput")
    smat_d = nc.dram_tensor("smat", [128, t_total * 128], dt.bfloat16, kind="ExternalInput")
    if has_b1:
        b1_d = nc.dram_tensor("b1bc", [P, hid], dt.float32, kind="ExternalInput")
    if has_b2:
        b2_d = nc.dram_tensor("b2bc", [P, out_f], dt.float32, kind="ExternalInput")
    out_d = nc.dram_tensor("out", [nshard, out_f], dt.float32, kind="ExternalOutput")

    rg = [list(range(ncores))]

    with tile.TileContext(nc) as tc:
        with (
            tc.tile_pool(name="const", bufs=1) as constp,
            tc.tile_pool(name="stage", bufs=1) as stagep,
            tc.tile_pool(name="dram", bufs=1, space="DRAM") as dramp,
            tc.tile_pool(name="gpool", bufs=4) as gpool,
            tc.tile_pool(name="spool", bufs=4) as spool,
        ):
            # ---- persistent SBUF ----
            xT_sb = constp.tile([P, kt, npad], dt.bfloat16)
            w1_sb = constp.tile([P, kt, hid], dt.bfloat16)
            w2_sb = constp.tile([hid, out_f], dt.bfloat16)
            idx_sb = constp.tile([128, t_total * 8], dt.int16)
            nc.sync.dma_start(out=xT_sb[:], in_=xT_d[:])
            nc.sync.dma_start(out=w1_sb[:], in_=w1_d[:])
            nc.sync.dma_start(out=w2_sb[:], in_=w2_d[:])
            nc.sync.dma_start(out=idx_sb[:], in_=idx_d[:])
            if has_b1:
                b1_sb = constp.tile([P, hid], dt.float32)
                nc.sync.dma_start(out=b1_sb[:], in_=b1_d[:])
            if has_b2:
                b2_sb = constp.tile([P, out_f], dt.float32)
                nc.sync.dma_start(out=b2_sb[:], in_=b2_d[:])

            zstage = stagep.tile([P, npad], dt.bfloat16)    # z = x@W1, node-major
            acc = stagep.tile([P, npad], dt.float32)        # f32 accumulator (both layers)
            hstage = stagep.tile([P, npad], dt.bfloat16)    # relu'd h, node-major
            aggT = stagep.tile([P, npad], dt.bfloat16)      # L2 agg, feat-major
            outstage = stagep.tile([P, nblk * out_f], dt.float32)


            def bts(i, sz):  # block tile slice
                return slice(i * sz, (i + 1) * sz)

            def valid(b):
                return lastv if b == nblk - 1 else P

            for _rep in range(reps):
                # ---- internal DRAM (gather tables / collective buffers) ----
                z_loc = dramp.tile([nshard, hid], dt.bfloat16, name=f"z_loc{_rep}")
                h_loc = dramp.tile([nshard, hid], dt.bfloat16, name=f"h_loc{_rep}")

                # ================= phase A: z = x @ W1 (node-major) =================
                for _ra in range(pm["A"]):
                  with tc.tile_pool(name=f"pA{_rep}_{_ra}", bufs=4, space="PSUM") as pA:
                    for t in range(nblk):
                        ps = pA.tile([P, hid], dt.float32, tag="psA")
                        for k in range(kt):
                            nc.tensor.matmul(
                                out=ps[:], lhsT=xT_sb[:, k, bts(t, P)], rhs=w1_sb[:, k, :],
                                start=(k == 0), stop=(k == kt - 1),
                            )
                        nc.scalar.copy(out=zstage[:, bts(t, hid)], in_=ps[:])
                        v = valid(t)
                        nc.sync.dma_start(out=z_loc[t * P : t * P + v, :], in_=zstage[:v, bts(t, hid)])

                for _rz in range(pm["AGZ"]):
                    z_full = dramp.tile([st["n"], hid], dt.bfloat16, addr_space="Shared",
                                        name=f"z_full{_rep}_{_rz}")
                    nc.gpsimd.collective_compute(
                        "AllGather", mybir.AluOpType.bypass, replica_groups=rg,
                        ins=[z_loc[:]], outs=[z_full[:]],
                    )

                # ================= edge aggregation (shared helper) =================
                def aggregate(layer, table, sub=0):
                    """layer 1: psum[slot,feat] (lhsT=S, rhs=G);
                    layer 2: psum[feat,slot] (lhsT=G, rhs=S).  Accumulate across the
                    two src-halves into `acc` (f32)."""
                    with tc.tile_pool(name=f"pB{layer}_{_rep}_{sub}", bufs=8, space="PSUM") as pB:
                        psd = {}
                        for ci, (h, cstart, cnt) in enumerate(chunks):
                            g = gpool.tile([P, CHUNK, hid], dt.bfloat16, tag="g", name=f"g{layer}")
                            src_ap = table[:] if h == 0 else table[halfsplit:, :]
                            nc.gpsimd.dma_gather(
                                g[:, :cnt, :], src_ap,
                                idx_sb[:, cstart * 8 : (cstart + cnt) * 8],
                                cnt * P, cnt * P, hid, single_packet=False,
                                queue_num=ci % nqueues,
                            )
                            s_sb = spool.tile([P, CHUNK, P], dt.bfloat16, tag="s", name=f"s{layer}")
                            nc.sync.dma_start(
                                out=s_sb[:, :cnt, :],
                                in_=smat_d[:, cstart * 128 : (cstart + cnt) * 128],
                            )
                            for p in range(cnt):
                                t = cstart + p
                                th, b, first, last = tile_info[t]
                                if first:
                                    psd[b] = pB.tile([P, P], dt.float32, tag="psB", name=f"ps{layer}")
                                if layer == 1:
                                    nc.tensor.matmul(out=psd[b][:, :hid], lhsT=s_sb[:, p, :], rhs=g[:, p, :],
                                                     start=first, stop=last)
                                else:
                                    nc.tensor.matmul(out=psd[b][:], lhsT=g[:, p, :], rhs=s_sb[:, p, :],
                                                     start=first, stop=last)
                                if last:
                                    if th == 0 or tiles_hb[0, b] == 0:
                                        nc.scalar.copy(out=acc[:, bts(b, P)], in_=psd[b][:])
                                    else:
                                        nc.vector.tensor_tensor(
                                            out=acc[:, bts(b, P)], in0=psd[b][:],
                                            in1=acc[:, bts(b, P)], op=Alu.add,
                                        )
                                    del psd[b]

                # ================= phase B: L1 aggregation + relu =================
                for _rb in range(pm["B"]):
                    aggregate(1, z_full, sub=_rb)
                    for b in range(nblk):
                        if has_b1:
                            nc.vector.tensor_tensor(out=acc[:, bts(b, P)], in0=acc[:, bts(b, P)],
                                                    in1=b1_sb[:], op=Alu.add)
                        nc.scalar.activation(out=hstage[:, bts(b, P)], in_=acc[:, bts(b, P)], func=Act.Relu)
                        v = valid(b)
                        nc.sync.dma_start(out=h_loc[b * P : b * P + v, :], in_=hstage[:v, bts(b, P)])

                for _rh in range(pm["AGH"]):
                    h_full = dramp.tile([st["n"], hid], dt.bfloat16, addr_space="Shared",
                                        name=f"h_full{_rep}_{_rh}")
                    nc.gpsimd.collective_compute(
                        "AllGather", mybir.AluOpType.bypass, replica_groups=rg,
                        ins=[h_loc[:]], outs=[h_full[:]],
                    )

                # ================= phase C: L2 aggregation (feat-major) =============
                for _rc in range(pm["C"]):
                    aggregate(2, h_full, sub=100 + _rc)
                    for b in range(nblk):
                        nc.scalar.copy(out=aggT[:, bts(b, P)], in_=acc[:, bts(b, P)])

                # ================= phase D: out = agg @ W2 (+ b2) ==================
                for _rd in range(pm["D"]):
                  with tc.tile_pool(name=f"pD{_rep}_{_rd}", bufs=4, space="PSUM") as pD:
                    for t in range(nblk):
                        ps = pD.tile([P, out_f], dt.float32, tag="psD")
                        nc.tensor.matmul(out=ps[:], lhsT=aggT[:, bts(t, P)], rhs=w2_sb[:],
                                         start=True, stop=True)
                        if has_b2:
                            nc.vector.tensor_tensor(out=outstage[:, bts(t, out_f)], in0=ps[:],
                                                    in1=b2_sb[:], op=Alu.add)
                        else:
                            nc.scalar.copy(out=outstage[:, bts(t, out_f)], in_=ps[:])
                        v = valid(t)
                        nc.sync.dma_start(out=out_d[t * P : t * P + v, :],
                                          in_=outstage[:v, bts(t, out_f)])

    nc.compile()
    return nc


# ----------------------------------------------------------------------------
# input packing
# ----------------------------------------------------------------------------

def pack_inputs(x, W1, b1, W2, b2, st, percore, perm):
    ncores, nshard, npad = st["ncores"], st["nshard"], st["npad"]
    kt = x.shape[1] // P
    hid = W1.shape[1]
    out_f = W2.shape[1]
    has_b1 = bool(np.any(b1))
    has_b2 = bool(np.any(b2))

    w1h = np.ascontiguousarray(
        W1.reshape(kt, P, hid).transpose(1, 0, 2)).astype(BF16)
    w2h = np.ascontiguousarray(W2).astype(BF16)

    xp = x[perm]  # balanced node order
    in_maps = []
    for c in range(ncores):
        xpad = np.zeros((npad, kt * P), np.float32)
        xpad[:nshard] = xp[c * nshard : (c + 1) * nshard]
        xT = np.ascontiguousarray(
            xpad.T.reshape(kt, P, npad).transpose(1, 0, 2)).astype(BF16)
        m = {
            "xT": xT, "w1": w1h, "w2": w2h,
            "idx": np.ascontiguousarray(percore["idx_w"][c]),
            "smat": np.ascontiguousarray(percore["smat"][c]),
        }
        if has_b1:
            m["b1bc"] = np.ascontiguousarray(np.broadcast_to(b1, (P, hid))).astype(np.float32)
        if has_b2:
            m["b2bc"] = np.ascontiguousarray(np.broadcast_to(b2, (P, out_f))).astype(np.float32)
        in_maps.append(m)
    return in_maps, has_b1, has_b2


# ----------------------------------------------------------------------------
# entry point
# ----------------------------------------------------------------------------

_CACHE = {}


def _run(x, edge_index, W1, b1, W2, b2, trace=False):
    from concourse.bass_utils import run_bass_kernel_spmd

    n = x.shape[0]
    st, percore, perm, pos = host_prep(edge_index, n, NCORES, HALFSPLIT, CHUNK)
    in_maps, has_b1, has_b2 = pack_inputs(x, W1, b1, W2, b2, st, percore, perm)

    key = (n, x.shape[1], W1.shape[1], W2.shape[1], st["t_total"],
           tuple(st["tiles_hb"].reshape(-1)), has_b1, has_b2)
    import os
    nq = int(os.environ.get("GCN_NQUEUES", "4"))
    key = key + (nq,)
    nc = _CACHE.get(key)
    if nc is None:
        nc = build_program(st, x.shape[1], W1.shape[1], W2.shape[1], has_b1, has_b2,
                           nqueues=nq)
        _CACHE[key] = nc

    res = run_bass_kernel_spmd(nc, in_maps, core_ids=list(range(NCORES)), trace=trace)
    outp = np.concatenate([res.results[c]["out"] for c in range(NCORES)], axis=0)
    out = np.empty_like(outp)
    out[perm] = outp  # undo balanced permutation
    return out.astype(np.float32), res


def kernel(x, edge_index, W1, b1, W2, b2):
    out, _ = _run(np.asarray(x, np.float32), np.asarray(edge_index),
                  np.asarray(W1, np.float32), np.asarray(b1, np.float32),
                  np.asarray(W2, np.float32), np.asarray(b2, np.float32))
    return out
